# revision 12
# baseline (speedup 1.0000x reference)
import os
import sys

import numpy as np

for _p in ("/opt/trn_rl_repo", "/root/.axon_site/_ro/trn_rl_repo"):
    if os.path.isdir(_p) and _p not in sys.path:
        sys.path.insert(0, _p)

import concourse.tile as tile
from concourse import bacc, mybir

# Problem: y = causal dilated conv1d (C=64->64, K=2, dilation=64) over x[16,64,16384],
# then f(y) = tanh(y)*sigmoid(y).  Sharded data-parallel over batch: 2 batches/core.
#
# All HBM I/O is fp16 (harness gate is rel_err < 2e-2; this lands ~4e-3), which
# halves DMA vs fp32.  The gate is then ACT-bound (2 LUT passes/elem, ~1ns/elem,
# no 16-bit accel), so some column blocks take "route B": one ACT pass
# t = tanh(y/2), then the DVE finishes  f = (t+t^2) * P(t^2)  with P(s) ~= 1/(1+s)
# (exact: tanh(y) = 2t/(1+t^2), sigmoid(y) = (1+t)/2).  The route A/B split
# balances the Scalar and Vector engines, and B pairs are processed FIRST with
# their DVE chains split in two and interleaved with A-block multiplies so the
# Vector FIFO never head-of-line-blocks the late cheap ops.
B, C, T = 16, 64, 16384
KERNEL = 2
DIL = 64
N_CORES = 8
B_PER = B // N_CORES  # 2
P = B_PER * C  # 128 partitions: batch 0 on 0..63, batch 1 on 64..127
BLK = 1536  # psum block (3 PSUM banks); 2 in flight of 8 banks
EDGE = 512  # first/last block size (short pipeline head/tail)
MM_FD = 512  # cols per matmul (PSUM bank limit for fp32 out)
F32 = mybir.dt.float32
IO_DTYPE = os.environ.get("KERNEL_IO_DTYPE", "float16")
BDEG = int(os.environ.get("KERNEL_BDEG", "2"))
# P(s) ~= 1/(1+s) on s in [0,1]; highest-degree first
BCOEF = {
    3: (-0.23548745, 0.68627748, -0.95078937, 0.99873732),  # minimax, err 1.3e-3
    2: (0.28671682, -0.7644057, 0.9776889),  # norm-weighted, err ~6e-3 on B cols
}[BDEG]
Tanh = mybir.ActivationFunctionType.Tanh
Sigmoid = mybir.ActivationFunctionType.Sigmoid
Mult = mybir.AluOpType.mult
Add = mybir.AluOpType.add


def _build_program():
    nc = bacc.Bacc("TRN2", target_bir_lowering=False, debug=False)
    iodt = getattr(mybir.dt, IO_DTYPE)
    x_in = nc.dram_tensor("x", [B_PER, C, T], iodt, kind="ExternalInput")
    # Host-preprocessed weights: wt[k] is the 128x128 block-diagonal stationary
    # matrix for tap k (two copies of w[:,:,k].T on the diagonal), so one K=128
    # matmul computes both batches' 64x64 channel mix.
    wt_in = nc.dram_tensor("wt", [KERNEL, P, P], iodt, kind="ExternalInput")
    y_out = nc.dram_tensor("y", [B_PER, C, T], iodt, kind="ExternalOutput")

    x_flat = x_in[:].flatten_outer_dims()  # [128, T]
    y_flat = y_out[:].flatten_outer_dims()  # [128, T]

    with tile.TileContext(nc) as tc:
        with (
            tc.tile_pool(name="wpool", bufs=1) as wpool,
            tc.tile_pool(name="xpool", bufs=3) as xpool,
            tc.tile_pool(name="apool", bufs=2) as apool,
            tc.tile_pool(name="bpool", bufs=2) as bpool,
            tc.tile_pool(name="opool", bufs=3) as opool,
            tc.tile_pool(name="psum", bufs=2, space="PSUM") as psumpool,
            tc.tile_pool(name="psdum", bufs=1, space="PSUM") as psdumpool,
        ):
            # Dummy-matmul stream: keeps chip-level activity monitors seeing a
            # busy PE.  Empirically, runs with a dense PE stream get nominal
            # ACT/DVE clocks (1.2/0.96 GHz); sparse-PE runs see ~0.83x both.
            wdum = wpool.tile([P, 128], iodt, tag="wdum")
            nc.vector.memset(wdum[:], 0.0078125)
            psd = psdumpool.tile([P, 256], F32, tag="psd")
            for _ in range(6):
                nc.tensor.matmul(out=psd[:, 0:128], lhsT=wdum[:],
                                 rhs=wdum[:], start=True, stop=True)

            # tiny weight loads first so the first matmuls are gated only on
            # the first x tile
            wblk = []
            for k in range(KERNEL):
                wk = wpool.tile([P, P], iodt, tag=f"w{k}")
                # gpsimd queue is idle at t=0: weights land immediately even
                # if the scheduler batches the sync-queue x fetches
                nc.gpsimd.dma_start(out=wk[:], in_=wt_in[k])
                wblk.append(wk)

            # prime the ACT tables: sigmoid_and_others contains both sigmoid
            # and tanh, so one table load serves every ACTIVATE below
            prime = wpool.tile([1, 2], F32, tag="prime")
            nc.vector.memset(prime[:], 0.0)
            nc.scalar.activation(out=prime[:, 0:1], in_=prime[:, 1:2], func=Sigmoid)

            def fetch_pair(pair):
                """DMA one pair (2*BLK cols + halo); pair 0..4."""
                t0 = EDGE + pair * 2 * BLK
                xt = xpool.tile([P, 2 * BLK + DIL], iodt, tag="xt")
                nc.sync.dma_start(out=xt[:], in_=x_flat[:, t0 - DIL : t0 + 2 * BLK])
                return xt, t0

            def conv_block(xt, base, width):
                """Matmuls for one block; xt col j = x[tile_t0 - DIL + j]."""
                ps = psumpool.tile([P, BLK], F32, tag="ps")
                for k in (1, 0):
                    for c in range(0, width, MM_FD):
                        nc.tensor.matmul(
                            out=ps[:, c : c + MM_FD],
                            lhsT=wblk[k][:],
                            rhs=xt[:, base + c + k * DIL : base + c + k * DIL + MM_FD],
                            start=(k == 1),
                            stop=(k == 0),
                        )
                # PE heartbeat (see dummy-stream note above)
                for _ in range(2):
                    nc.tensor.matmul(out=psd[:, 0:128], lhsT=wdum[:],
                                     rhs=wdum[:], start=True, stop=True)
                return ps

            def conv_pair_b(pair):
                """Route B front half: conv + one tanh(y/2) pass per block."""
                xt, t0 = fetch_pair(pair)
                t = bpool.tile([P, 2 * BLK], iodt, tag="bt")
                for half in range(2):
                    ps = conv_block(xt, half * BLK, BLK)
                    nc.scalar.activation(
                        out=t[:, half * BLK : (half + 1) * BLK], in_=ps[:],
                        func=Tanh, scale=0.5,
                    )
                # extra heartbeats: the PE would otherwise idle >3.4us during
                # this pair's long DVE chain and HAM-throttle to 1.2GHz,
                # making the next pair's psum fill slow enough to stall ACT
                for _ in range(8):
                    nc.tensor.matmul(out=psd[:, 0:128], lhsT=wdum[:],
                                     rhs=wdum[:], start=True, stop=True)
                return t, t0

            def chain_b_part1(t, width):
                """s=t^2, m=t+s, u=c_hi*s+c_next, v=u*s."""
                c = BCOEF
                s = bpool.tile([P, width], iodt, tag="bs")
                nc.vector.tensor_mul(s[:], t[:, 0:width], t[:, 0:width])
                m = bpool.tile([P, width], iodt, tag="bm")
                nc.vector.tensor_add(m[:], t[:, 0:width], s[:])
                u = bpool.tile([P, width], iodt, tag="bu")
                nc.vector.tensor_scalar(
                    out=u[:], in0=s[:], scalar1=float(c[0]), scalar2=float(c[1]),
                    op0=Mult, op1=Add,
                )
                v = bpool.tile([P, width], iodt, tag="bv")
                nc.vector.tensor_mul(v[:], u[:], s[:])
                return s, m, v

            def chain_b_part2(s, m, v, width, t0):
                c = BCOEF
                if BDEG == 3:
                    w_ = bpool.tile([P, width], iodt, tag="bw")
                    nc.vector.tensor_scalar_add(w_[:], v[:], float(c[2]))
                    p_ = bpool.tile([P, width], iodt, tag="bp")
                    nc.vector.tensor_mul(p_[:], w_[:], s[:])
                    q = bpool.tile([P, width], iodt, tag="bq")
                    nc.vector.tensor_scalar_add(q[:], p_[:], float(c[3]))
                else:
                    q = bpool.tile([P, width], iodt, tag="bq")
                    nc.vector.tensor_scalar_add(q[:], v[:], float(c[2]))
                ot = opool.tile([P, width], iodt, tag="ot")
                nc.vector.tensor_mul(ot[:], q[:], m[:])
                nc.gpsimd.dma_start(out=y_flat[:, t0 : t0 + width], in_=ot[:])

            def conv_pair_a(pair):
                """Route A front half: conv + tanh + sigmoid per block."""
                xt, t0 = fetch_pair(pair)
                th = apool.tile([P, 2 * BLK], iodt, tag="th")
                sg = apool.tile([P, 2 * BLK], iodt, tag="sg")
                for half in range(2):
                    ps = conv_block(xt, half * BLK, BLK)
                    lo, hi = half * BLK, (half + 1) * BLK
                    nc.scalar.activation(out=th[:, lo:hi], in_=ps[:], func=Tanh)
                    nc.scalar.activation(out=sg[:, lo:hi], in_=ps[:], func=Sigmoid)
                return th, sg, t0

            def mul_store_a(th, sg, width, t0, store_eng=None):
                ot = opool.tile([P, width], iodt, tag="ot")
                nc.vector.tensor_mul(ot[:], th[:, 0:width], sg[:, 0:width])
                (store_eng or nc.gpsimd).dma_start(
                    out=y_flat[:, t0 : t0 + width], in_=ot[:])

            def conv_edge(t0):
                """One EDGE-wide route-A block at t0 (0 or T-EDGE)."""
                xt = xpool.tile([P, EDGE + DIL], iodt, tag="xt_e")
                if t0 == 0:
                    nc.vector.memset(xt[:, 0:DIL], 0.0)
                    nc.sync.dma_start(out=xt[:, DIL:], in_=x_flat[:, 0:EDGE])
                else:
                    nc.sync.dma_start(out=xt[:], in_=x_flat[:, t0 - DIL : t0 + EDGE])
                ps = conv_block(xt, 0, EDGE)
                th = apool.tile([P, EDGE], iodt, tag="th_e")
                sg = apool.tile([P, EDGE], iodt, tag="sg_e")
                nc.scalar.activation(out=th[:, 0:EDGE], in_=ps[:, 0:EDGE], func=Tanh)
                nc.scalar.activation(out=sg[:, 0:EDGE], in_=ps[:, 0:EDGE], func=Sigmoid)
                return th, sg

            # Processing order: B pairs first (their long DVE chains start
            # early and drain under later A-pair ACT work), chains split so
            # cheap A multiplies interleave into the Vector FIFO, edges last.
            W2 = 2 * BLK
            the0, sge0 = conv_edge(0)                       # ACT: 2 passes
            thA0, sgA0, t0A0 = conv_pair_a(0)               # ACT: 4 passes
            mul_store_a(the0, sge0, EDGE, 0)                # DVE: 1 op
            tB1, t0B1 = conv_pair_b(1)                      # ACT: 2 passes
            mul_store_a(thA0, sgA0, W2, t0A0)               # DVE: 1 op
            c1 = chain_b_part1(tB1, W2)                     # DVE: 4 ops
            tB3, t0B3 = conv_pair_b(3)                      # ACT: 2 passes
            chain_b_part2(*c1, W2, t0B1)                    # DVE: 2-4 ops
            thA2, sgA2, t0A2 = conv_pair_a(2)               # ACT: 4 passes
            c3 = chain_b_part1(tB3, W2)                     # DVE: 4 ops
            mul_store_a(thA2, sgA2, W2, t0A2)               # DVE: 1 op
            thA4, sgA4, t0A4 = conv_pair_a(4)               # ACT: 4 passes
            chain_b_part2(*c3, W2, t0B3)                    # DVE: 2-4 ops
            mul_store_a(thA4, sgA4, W2, t0A4, nc.sync)      # DVE: 1 op
            the1, sge1 = conv_edge(T - EDGE)                # ACT: 2 passes
            mul_store_a(the1, sge1, EDGE, T - EDGE, nc.sync)  # DVE: 1 op
    nc.finalize()
    return nc


def _host_weights(w: np.ndarray, np_dtype) -> np.ndarray:
    wt = np.zeros((KERNEL, P, P), dtype=np_dtype)
    for k in range(KERNEL):
        wTk = np.ascontiguousarray(w[:, :, k].T).astype(np_dtype)  # [ci, co]
        for b in range(B_PER):
            wt[k, b * C : (b + 1) * C, b * C : (b + 1) * C] = wTk
    return wt


def _ensure_ntff_hook():
    """Recreate the antenv.axon_hooks NTFF profiling shim if the image lacks it."""
    import types

    try:
        import antenv.axon_hooks  # noqa: F401

        return
    except ImportError:
        pass
    import antenv

    mod = types.ModuleType("antenv.axon_hooks")
    _h = {"hook": None}
    mod.set_axon_ntff_profile_hook = lambda h: _h.__setitem__("hook", h)
    mod.get_axon_ntff_profile_hook = lambda: _h["hook"]
    sys.modules["antenv.axon_hooks"] = mod
    antenv.axon_hooks = mod
    try:
        from trn_agent_boot.trn_boot import _ntff_profile_via_ctypes

        hook = _ntff_profile_via_ctypes("/opt/axon/libaxon_pjrt.so")
        if hook is not None:
            mod.set_axon_ntff_profile_hook(hook)
    except Exception as e:  # degrade to no-trace rather than crash
        print(f"ntff hook setup failed: {e}", file=sys.stderr)


def _run_spmd(x: np.ndarray, w: np.ndarray, trace: bool = False):
    from concourse import bass_utils
    from concourse.bass_utils import run_bass_kernel_spmd

    if trace:
        _ensure_ntff_hook()
        bass_utils.upload_artifacts = lambda tmpdir: tmpdir

    nc = _build_program()
    if IO_DTYPE == "bfloat16":
        import ml_dtypes

        np_dtype = ml_dtypes.bfloat16
    else:
        np_dtype = np.float16
    xio = np.ascontiguousarray(x.astype(np_dtype))
    wt = _host_weights(w, np_dtype)
    in_maps = [
        {"x": np.ascontiguousarray(xio[i * B_PER : (i + 1) * B_PER]), "wt": wt}
        for i in range(N_CORES)
    ]
    kwargs = {}
    if trace:
        import tempfile

        os.makedirs("/tmp/kernel_trace", exist_ok=True)
        kwargs["tmpdir"] = tempfile.mkdtemp(dir="/tmp/kernel_trace")
    res = run_bass_kernel_spmd(nc, in_maps, list(range(N_CORES)), trace=trace, **kwargs)
    y = np.concatenate(
        [res.results[i]["y"].astype(np.float32) for i in range(N_CORES)], axis=0
    )
    return y, res


def kernel(x: np.ndarray, w: np.ndarray) -> np.ndarray:
    x = np.ascontiguousarray(np.asarray(x, dtype=np.float32))
    w = np.ascontiguousarray(np.asarray(w, dtype=np.float32))
    trace = os.environ.get("KERNEL_TRACE", "0") == "1"
    y, res = _run_spmd(x, w, trace=trace)
    if trace:
        global LAST_RESULTS
        LAST_RESULTS = res
    return y


LAST_RESULTS = None


# revision 13
# speedup vs baseline: 1.2346x; 1.2346x over previous
import os
import sys

import numpy as np

for _p in ("/opt/trn_rl_repo", "/root/.axon_site/_ro/trn_rl_repo"):
    if os.path.isdir(_p) and _p not in sys.path:
        sys.path.insert(0, _p)

import concourse.tile as tile
from concourse import bacc, mybir

# Problem: y = causal dilated conv1d (C=64->64, K=2, dilation=64) over x[16,64,16384],
# then f(y) = tanh(y)*sigmoid(y).  Sharded data-parallel over batch: 2 batches/core.
#
# All HBM I/O is fp16 (harness gate is rel_err < 2e-2; this lands ~4e-3), which
# halves DMA vs fp32.  The gate is then ACT-bound (2 LUT passes/elem, ~1ns/elem,
# no 16-bit accel), so some column blocks take "route B": one ACT pass
# t = tanh(y/2), then the DVE finishes  f = (t+t^2) * P(t^2)  with P(s) ~= 1/(1+s)
# (exact: tanh(y) = 2t/(1+t^2), sigmoid(y) = (1+t)/2).  The route A/B split
# balances the Scalar and Vector engines, and B pairs are processed FIRST with
# their DVE chains split in two and interleaved with A-block multiplies so the
# Vector FIFO never head-of-line-blocks the late cheap ops.
B, C, T = 16, 64, 16384
KERNEL = 2
DIL = 64
N_CORES = 8
B_PER = B // N_CORES  # 2
P = B_PER * C  # 128 partitions: batch 0 on 0..63, batch 1 on 64..127
BLK = 1536  # psum block (3 PSUM banks); 2 in flight of 8 banks
EDGE = 512  # first/last block size (short pipeline head/tail)
MM_FD = 512  # cols per matmul (PSUM bank limit for fp32 out)
F32 = mybir.dt.float32
IO_DTYPE = os.environ.get("KERNEL_IO_DTYPE", "float16")
BDEG = int(os.environ.get("KERNEL_BDEG", "2"))
# P(s) ~= 1/(1+s) on s in [0,1]; highest-degree first
BCOEF = {
    3: (-0.23548745, 0.68627748, -0.95078937, 0.99873732),  # minimax, err 1.3e-3
    2: (0.28671682, -0.7644057, 0.9776889),  # norm-weighted, err ~6e-3 on B cols
}[BDEG]
Tanh = mybir.ActivationFunctionType.Tanh
Sigmoid = mybir.ActivationFunctionType.Sigmoid
Mult = mybir.AluOpType.mult
Add = mybir.AluOpType.add


def _build_program():
    nc = bacc.Bacc("TRN2", target_bir_lowering=False, debug=False)
    iodt = getattr(mybir.dt, IO_DTYPE)
    x_in = nc.dram_tensor("x", [B_PER, C, T], iodt, kind="ExternalInput")
    # Host-preprocessed weights: wt[k] is the 128x128 block-diagonal stationary
    # matrix for tap k (two copies of w[:,:,k].T on the diagonal), so one K=128
    # matmul computes both batches' 64x64 channel mix.
    wt_in = nc.dram_tensor("wt", [KERNEL, P, P], iodt, kind="ExternalInput")
    y_out = nc.dram_tensor("y", [B_PER, C, T], iodt, kind="ExternalOutput")

    x_flat = x_in[:].flatten_outer_dims()  # [128, T]
    y_flat = y_out[:].flatten_outer_dims()  # [128, T]

    with tile.TileContext(nc) as tc:
        with (
            tc.tile_pool(name="wpool", bufs=1) as wpool,
            tc.tile_pool(name="xpool", bufs=3) as xpool,
            tc.tile_pool(name="apool", bufs=2) as apool,
            tc.tile_pool(name="bpool", bufs=2) as bpool,
            tc.tile_pool(name="opool", bufs=3) as opool,
            tc.tile_pool(name="psum", bufs=2, space="PSUM") as psumpool,
            tc.tile_pool(name="psdum", bufs=1, space="PSUM") as psdumpool,
        ):
            # Dummy-matmul stream: keeps chip-level activity monitors seeing a
            # busy PE.  Empirically, runs with a dense PE stream get nominal
            # ACT/DVE clocks (1.2/0.96 GHz); sparse-PE runs see ~0.83x both.
            wdum = wpool.tile([P, 128], iodt, tag="wdum")
            nc.vector.memset(wdum[:], 0.0078125)
            psd = psdumpool.tile([P, 256], F32, tag="psd")
            for _ in range(12):
                nc.tensor.matmul(out=psd[:, 0:128], lhsT=wdum[:],
                                 rhs=wdum[:], start=True, stop=True)

            # tiny weight loads first so the first matmuls are gated only on
            # the first x tile
            wblk = []
            for k in range(KERNEL):
                wk = wpool.tile([P, P], iodt, tag=f"w{k}")
                # gpsimd queue is idle at t=0: weights land immediately even
                # if the scheduler batches the sync-queue x fetches
                nc.gpsimd.dma_start(out=wk[:], in_=wt_in[k])
                wblk.append(wk)

            # prime the ACT tables: sigmoid_and_others contains both sigmoid
            # and tanh, so one table load serves every ACTIVATE below
            prime = wpool.tile([1, 2], F32, tag="prime")
            nc.vector.memset(prime[:], 0.0)
            nc.scalar.activation(out=prime[:, 0:1], in_=prime[:, 1:2], func=Sigmoid)

            def fetch_pair(pair):
                """DMA one pair (2*BLK cols + halo); pair 0..4."""
                t0 = EDGE + pair * 2 * BLK
                xt = xpool.tile([P, 2 * BLK + DIL], iodt, tag="xt")
                nc.sync.dma_start(out=xt[:], in_=x_flat[:, t0 - DIL : t0 + 2 * BLK])
                return xt, t0

            def conv_block(xt, base, width):
                """Matmuls for one block; xt col j = x[tile_t0 - DIL + j]."""
                ps = psumpool.tile([P, BLK], F32, tag="ps")
                for k in (1, 0):
                    for c in range(0, width, MM_FD):
                        nc.tensor.matmul(
                            out=ps[:, c : c + MM_FD],
                            lhsT=wblk[k][:],
                            rhs=xt[:, base + c + k * DIL : base + c + k * DIL + MM_FD],
                            start=(k == 1),
                            stop=(k == 0),
                        )
                # PE heartbeat (see dummy-stream note above)
                for _ in range(2):
                    nc.tensor.matmul(out=psd[:, 0:128], lhsT=wdum[:],
                                     rhs=wdum[:], start=True, stop=True)
                return ps

            def conv_pair_b(pair):
                """Route B front half: conv + one tanh(y/2) pass per block."""
                xt, t0 = fetch_pair(pair)
                t = bpool.tile([P, 2 * BLK], iodt, tag="bt")
                for half in range(2):
                    ps = conv_block(xt, half * BLK, BLK)
                    nc.scalar.activation(
                        out=t[:, half * BLK : (half + 1) * BLK], in_=ps[:],
                        func=Tanh, scale=0.5,
                    )
                # extra heartbeats: the PE would otherwise idle >3.4us during
                # this pair's long DVE chain and HAM-throttle to 1.2GHz,
                # making the next pair's psum fill slow enough to stall ACT
                for _ in range(8):
                    nc.tensor.matmul(out=psd[:, 0:128], lhsT=wdum[:],
                                     rhs=wdum[:], start=True, stop=True)
                return t, t0

            def chain_b_part1(t, width):
                """s=t^2, m=t+s, u=c_hi*s+c_next, v=u*s."""
                c = BCOEF
                s = bpool.tile([P, width], iodt, tag="bs")
                nc.vector.tensor_mul(s[:], t[:, 0:width], t[:, 0:width])
                m = bpool.tile([P, width], iodt, tag="bm")
                nc.vector.tensor_add(m[:], t[:, 0:width], s[:])
                u = bpool.tile([P, width], iodt, tag="bu")
                nc.vector.tensor_scalar(
                    out=u[:], in0=s[:], scalar1=float(c[0]), scalar2=float(c[1]),
                    op0=Mult, op1=Add,
                )
                v = bpool.tile([P, width], iodt, tag="bv")
                nc.vector.tensor_mul(v[:], u[:], s[:])
                return s, m, v

            def chain_b_part2(s, m, v, width, t0):
                c = BCOEF
                if BDEG == 3:
                    w_ = bpool.tile([P, width], iodt, tag="bw")
                    nc.vector.tensor_scalar_add(w_[:], v[:], float(c[2]))
                    p_ = bpool.tile([P, width], iodt, tag="bp")
                    nc.vector.tensor_mul(p_[:], w_[:], s[:])
                    q = bpool.tile([P, width], iodt, tag="bq")
                    nc.vector.tensor_scalar_add(q[:], p_[:], float(c[3]))
                else:
                    q = bpool.tile([P, width], iodt, tag="bq")
                    nc.vector.tensor_scalar_add(q[:], v[:], float(c[2]))
                ot = opool.tile([P, width], iodt, tag="ot")
                nc.vector.tensor_mul(ot[:], q[:], m[:])
                nc.gpsimd.dma_start(out=y_flat[:, t0 : t0 + width], in_=ot[:])

            def conv_pair_a(pair):
                """Route A front half: conv + tanh + sigmoid per block."""
                xt, t0 = fetch_pair(pair)
                th = apool.tile([P, 2 * BLK], iodt, tag="th")
                sg = apool.tile([P, 2 * BLK], iodt, tag="sg")
                for half in range(2):
                    ps = conv_block(xt, half * BLK, BLK)
                    lo, hi = half * BLK, (half + 1) * BLK
                    nc.scalar.activation(out=th[:, lo:hi], in_=ps[:], func=Tanh)
                    nc.scalar.activation(out=sg[:, lo:hi], in_=ps[:], func=Sigmoid)
                return th, sg, t0

            def mul_store_a(th, sg, width, t0, store_eng=None):
                ot = opool.tile([P, width], iodt, tag="ot")
                nc.vector.tensor_mul(ot[:], th[:, 0:width], sg[:, 0:width])
                (store_eng or nc.gpsimd).dma_start(
                    out=y_flat[:, t0 : t0 + width], in_=ot[:])

            def conv_edge(t0):
                """One EDGE-wide route-A block at t0 (0 or T-EDGE)."""
                xt = xpool.tile([P, EDGE + DIL], iodt, tag="xt_e")
                if t0 == 0:
                    nc.vector.memset(xt[:, 0:DIL], 0.0)
                    nc.sync.dma_start(out=xt[:, DIL:], in_=x_flat[:, 0:EDGE])
                else:
                    nc.sync.dma_start(out=xt[:], in_=x_flat[:, t0 - DIL : t0 + EDGE])
                ps = conv_block(xt, 0, EDGE)
                th = apool.tile([P, EDGE], iodt, tag="th_e")
                sg = apool.tile([P, EDGE], iodt, tag="sg_e")
                nc.scalar.activation(out=th[:, 0:EDGE], in_=ps[:, 0:EDGE], func=Tanh)
                nc.scalar.activation(out=sg[:, 0:EDGE], in_=ps[:, 0:EDGE], func=Sigmoid)
                return th, sg

            # Processing order: B pairs first (their long DVE chains start
            # early and drain under later A-pair ACT work), chains split so
            # cheap A multiplies interleave into the Vector FIFO, edges last.
            W2 = 2 * BLK
            the0, sge0 = conv_edge(0)                       # ACT: 2 passes
            tB1, t0B1 = conv_pair_b(1)                      # ACT: 2 passes
            mul_store_a(the0, sge0, EDGE, 0)                # DVE: 1 op
            c1 = chain_b_part1(tB1, W2)                     # DVE: 4 ops
            tB3, t0B3 = conv_pair_b(3)                      # ACT: 2 passes
            chain_b_part2(*c1, W2, t0B1)                    # DVE: 2-4 ops
            thA0, sgA0, t0A0 = conv_pair_a(0)               # ACT: 4 passes
            c3 = chain_b_part1(tB3, W2)                     # DVE: 4 ops
            mul_store_a(thA0, sgA0, W2, t0A0)               # DVE: 1 op
            thA2, sgA2, t0A2 = conv_pair_a(2)               # ACT: 4 passes
            chain_b_part2(*c3, W2, t0B3)                    # DVE: 2-4 ops
            mul_store_a(thA2, sgA2, W2, t0A2)               # DVE: 1 op
            thA4, sgA4, t0A4 = conv_pair_a(4)               # ACT: 4 passes
            mul_store_a(thA4, sgA4, W2, t0A4)               # DVE: 1 op
            the1, sge1 = conv_edge(T - EDGE)                # ACT: 2 passes
            mul_store_a(the1, sge1, EDGE, T - EDGE)         # DVE: 1 op
    nc.finalize()
    return nc


def _host_weights(w: np.ndarray, np_dtype) -> np.ndarray:
    wt = np.zeros((KERNEL, P, P), dtype=np_dtype)
    for k in range(KERNEL):
        wTk = np.ascontiguousarray(w[:, :, k].T).astype(np_dtype)  # [ci, co]
        for b in range(B_PER):
            wt[k, b * C : (b + 1) * C, b * C : (b + 1) * C] = wTk
    return wt


def _ensure_ntff_hook():
    """Recreate the antenv.axon_hooks NTFF profiling shim if the image lacks it."""
    import types

    try:
        import antenv.axon_hooks  # noqa: F401

        return
    except ImportError:
        pass
    import antenv

    mod = types.ModuleType("antenv.axon_hooks")
    _h = {"hook": None}
    mod.set_axon_ntff_profile_hook = lambda h: _h.__setitem__("hook", h)
    mod.get_axon_ntff_profile_hook = lambda: _h["hook"]
    sys.modules["antenv.axon_hooks"] = mod
    antenv.axon_hooks = mod
    try:
        from trn_agent_boot.trn_boot import _ntff_profile_via_ctypes

        hook = _ntff_profile_via_ctypes("/opt/axon/libaxon_pjrt.so")
        if hook is not None:
            mod.set_axon_ntff_profile_hook(hook)
    except Exception as e:  # degrade to no-trace rather than crash
        print(f"ntff hook setup failed: {e}", file=sys.stderr)


def _run_spmd(x: np.ndarray, w: np.ndarray, trace: bool = False):
    from concourse import bass_utils
    from concourse.bass_utils import run_bass_kernel_spmd

    if trace:
        _ensure_ntff_hook()
        bass_utils.upload_artifacts = lambda tmpdir: tmpdir

    nc = _build_program()
    if IO_DTYPE == "bfloat16":
        import ml_dtypes

        np_dtype = ml_dtypes.bfloat16
    else:
        np_dtype = np.float16
    xio = np.ascontiguousarray(x.astype(np_dtype))
    wt = _host_weights(w, np_dtype)
    in_maps = [
        {"x": np.ascontiguousarray(xio[i * B_PER : (i + 1) * B_PER]), "wt": wt}
        for i in range(N_CORES)
    ]
    kwargs = {}
    if trace:
        import tempfile

        os.makedirs("/tmp/kernel_trace", exist_ok=True)
        kwargs["tmpdir"] = tempfile.mkdtemp(dir="/tmp/kernel_trace")
    res = run_bass_kernel_spmd(nc, in_maps, list(range(N_CORES)), trace=trace, **kwargs)
    y = np.concatenate(
        [res.results[i]["y"].astype(np.float32) for i in range(N_CORES)], axis=0
    )
    return y, res


def kernel(x: np.ndarray, w: np.ndarray) -> np.ndarray:
    x = np.ascontiguousarray(np.asarray(x, dtype=np.float32))
    w = np.ascontiguousarray(np.asarray(w, dtype=np.float32))
    trace = os.environ.get("KERNEL_TRACE", "0") == "1"
    y, res = _run_spmd(x, w, trace=trace)
    if trace:
        global LAST_RESULTS
        LAST_RESULTS = res
    return y


LAST_RESULTS = None
